# revision 3
# baseline (speedup 1.0000x reference)
"""RNN-T Joiner kernel for 8 Trainium2 NeuronCores.

out[b,t,u,:] = tanh(enc[b,t,:] + pred[b,u,:]) @ W.T + b

Sharding: data-parallel over t (400 -> 50 per core). Each core:
  - DVE/GPSIMD: broadcast-add encT[:,t] + predT[:,u] (f32 in, bf16 out)
  - ACT: tanh in place (bf16)
  - PE:  psum[cells, v] += logitT[c, cells].T @ WT[c, v]  (bf16, N=512)
  - output pass, two alternating paths so ACT shares the load with DVE:
      1-pass: DVE tensor_tensor psum(f32)+bias_f32 -> osb bf16   (PSUM 1x)
      2-pass: ACT copy psum->osb bf16, DVE bf16+bias_bf16 in place (2x_1P)
  - DMA: one 500-cell (500KB DRAM span, 4KB/partition) store per psum
    quad-tile on the sync HWDGE queue
  - host: upcast bf16 -> f32 during the gather

Engine budget per core (measured rates): PE 640 MM x ~216ns = ~138us is the
critical path. DVE ~115us (80 one-pass copies @0.68us, 80 bf16 adds @0.37us,
~30 producer adds @1.1us), GPSIMD ~110us (50 adds @2.2us two-input floor),
ACT ~123us (80 tanh @0.95us + 80 psum copies @0.59us), DMA out bf16 20.5MB
~50us. Finer BLK=5 blocks cut the startup serial chain (consts DMA -> first
add -> tanh) to ~8us.
"""

import sys

sys.path.insert(0, "/opt/trn_rl_repo")

import ml_dtypes
import numpy as np

import concourse.bass as bass
import concourse.bacc as bacc
import concourse.mybir as mybir
from concourse.tile import TileContext
from concourse.bass_utils import run_bass_kernel_spmd

B, T, U, C, V = 4, 400, 100, 512, 512
NCORES = 8
TS = T // NCORES  # 50 t per core
P = 128
CK = C // P  # 4 chunks of the contraction dim
CELLS = TS * U  # 5000 cells (t,u) per batch per core
BLK = 5  # logit blocks per batch
BCELLS = CELLS // BLK  # 1000
BT = TS // BLK  # 10 t per block
TILE = 125  # uniform cell tile (8 per block); m<128 costs no PE streaming
NT = BCELLS // TILE  # 8
F32 = mybir.dt.float32
BF16 = mybir.dt.bfloat16

# f32 consts blob (bias, enc, pred) -- f32 keeps the broadcast adds in DVE
# 1x mode (bf16 broadcast APs fall out of 2x_1P and run *slower*)
BIAS_OFF = 0
ENC_OFF = BIAS_OFF + V  # [ck, b, t]
PRED_OFF = ENC_OFF + CK * B * TS  # [ck, b, u]
NCOL_F = PRED_OFF + CK * B * U  # 2912
# bf16 consts blob: just W, pre-transposed [p, ck, v]
NCOL_W = CK * V  # 2048

# producer add engine per unit (b*BLK+blk)*CK+ck: ~30 on DVE, rest GPSIMD
N_UNITS = B * BLK * CK  # 80


def _add_engine(unit):
    return "D" if unit % 8 in (0, 3, 6) else "G"


_cache = {}


def _build():
    # Bacc (not raw Bass): its compile() runs generate_event_semaphores,
    # which splits >1-wait sync conditions that walrus rejects.
    nc = bacc.Bacc("TRN2", target_bir_lowering=False, debug=False)
    cf = nc.declare_dram_parameter("cf", [P, NCOL_F], F32, isOutput=False)
    cw = nc.declare_dram_parameter("cw", [P, NCOL_W], BF16, isOutput=False)
    out = nc.declare_dram_parameter("out", [B, TS, U, V], BF16, isOutput=True)

    with TileContext(nc) as tc:
        with (
            tc.tile_pool(name="consts", bufs=1) as cpool,
            tc.tile_pool(name="logit", bufs=3) as logit_pool,
            tc.tile_pool(name="osb", bufs=3) as out_pool,
            tc.tile_pool(name="psum", bufs=2, space="PSUM") as psum_pool,
        ):
            csf = cpool.tile([P, NCOL_F], F32, tag="csf")
            csw = cpool.tile([P, NCOL_W], BF16, tag="csw")
            nc.sync.dma_start(out=csf, in_=cf.ap())
            nc.sync.dma_start(out=csw, in_=cw.ap())

            wview = csw[:].rearrange("p (ck v) -> p ck v", ck=CK)
            bias_f32 = csf[:, BIAS_OFF : BIAS_OFF + V]
            eview = csf[:, ENC_OFF : ENC_OFF + CK * B * TS].rearrange(
                "p (ck b t) -> p ck b t", ck=CK, b=B
            )
            pview = csf[:, PRED_OFF : PRED_OFF + CK * B * U].rearrange(
                "p (ck b u) -> p ck b u", ck=CK, b=B
            )
            # bf16 bias copy for the 2-pass output path (2x_1P needs step-1
            # bf16 operands)
            bias_bf = cpool.tile([P, V], BF16, tag="bias_bf")
            nc.vector.tensor_copy(out=bias_bf[:], in_=bias_f32)

            tile_idx = 0
            for b in range(B):
                ob = out.ap()[b].rearrange("t u v -> (t u) v")  # [5000, 512]
                for blk in range(BLK):
                    t0 = blk * BT
                    c0 = blk * BCELLS
                    lg = []
                    for ck in range(CK):
                        lgt = logit_pool.tile([P, BT, U], BF16, tag=f"lg{ck}")
                        e_col = (
                            eview[:, ck, b, t0 : t0 + BT]
                            .unsqueeze(2)
                            .broadcast_to([P, BT, U])
                        )
                        p_row = (
                            pview[:, ck, b, :]
                            .unsqueeze(1)
                            .broadcast_to([P, BT, U])
                        )
                        unit = (b * BLK + blk) * CK + ck
                        eng = nc.vector if _add_engine(unit) == "D" else nc.gpsimd
                        eng.tensor_add(out=lgt[:], in0=e_col, in1=p_row)
                        nc.scalar.activation(
                            out=lgt[:],
                            in_=lgt[:],
                            func=mybir.ActivationFunctionType.Tanh,
                        )
                        lg.append(lgt[:].rearrange("p t u -> p (t u)"))
                    for q in range(2):
                        ps = psum_pool.tile([P, NT // 2, V], F32, tag="ps")
                        osb = out_pool.tile([P, NT // 2, V], BF16, tag="osb")
                        for j in range(NT // 2):
                            s = (q * (NT // 2) + j) * TILE
                            for ck in range(CK):
                                nc.tensor.matmul(
                                    ps[:TILE, j, :],
                                    lhsT=lg[ck][:, s : s + TILE],
                                    rhs=wview[:, ck, :],
                                    start=(ck == 0),
                                    stop=(ck == CK - 1),
                                )
                            if tile_idx % 2 == 0:
                                # 1-pass: DVE psum+bias (PSUM caps TT at 1x)
                                nc.vector.tensor_add(
                                    out=osb[:TILE, j, :],
                                    in0=ps[:TILE, j, :],
                                    in1=bias_f32[:TILE],
                                )
                            else:
                                # 2-pass: ACT copy (psum 2x-ish), DVE bf16 add
                                nc.scalar.activation(
                                    out=osb[:TILE, j, :],
                                    in_=ps[:TILE, j, :],
                                    func=mybir.ActivationFunctionType.Copy,
                                )
                                nc.vector.tensor_add(
                                    out=osb[:TILE, j, :],
                                    in0=osb[:TILE, j, :],
                                    in1=bias_bf[:TILE],
                                )
                            tile_idx += 1
                        oq = ob[
                            c0 + q * (NT // 2) * TILE : c0 + (q + 1) * (NT // 2) * TILE
                        ].rearrange("(j p) v -> p j v", p=TILE)
                        nc.sync.dma_start(out=oq, in_=osb[:TILE])
    nc.compile()
    return nc


def _install_ntff_hook():
    """This image's antenv lacks axon_hooks, so bass_utils' trace=True path
    can't find the NTFF profile hook. Inject the module and wire the ctypes
    hook from trn_boot against the axon PJRT .so."""
    if "antenv.axon_hooks" in sys.modules:
        return
    import types

    holder = [None]
    mod = types.ModuleType("antenv.axon_hooks")
    mod.set_axon_ntff_profile_hook = lambda h: holder.__setitem__(0, h)
    mod.get_axon_ntff_profile_hook = lambda: holder[0]
    sys.modules["antenv.axon_hooks"] = mod
    try:
        sys.path.insert(0, "/root/.axon_site/trn_agent_boot")
        from trn_boot import _ntff_profile_via_ctypes

        mod.set_axon_ntff_profile_hook(
            _ntff_profile_via_ctypes("/opt/axon/libaxon_pjrt.so")
        )
    except Exception as e:  # degrade to no tracing
        print(f"NTFF hook install failed: {e}", file=sys.stderr)


def _run(in_maps, trace=False, tmpdir=None):
    if "nc" not in _cache:
        _cache["nc"] = _build()
    if trace:
        _install_ntff_hook()
    return run_bass_kernel_spmd(
        _cache["nc"], in_maps, list(range(NCORES)), trace=trace, tmpdir=tmpdir
    )


def make_in_maps(encoder_out, predictor_out, W, b):
    encoder_out = np.asarray(encoder_out, dtype=np.float32)
    predictor_out = np.asarray(predictor_out, dtype=np.float32)
    W = np.asarray(W, dtype=np.float32)
    b = np.asarray(b, dtype=np.float32)

    # [p, ck, v] <- W[v, ck*P+p]
    wpack = np.ascontiguousarray(
        W.reshape(V, CK, P).transpose(2, 1, 0).reshape(P, CK * V)
    ).astype(ml_dtypes.bfloat16)

    base = np.empty((P, NCOL_F), np.float32)
    base[:, BIAS_OFF : BIAS_OFF + V] = np.broadcast_to(b, (P, V))
    # [p, ck, b, u] <- pred[b, u, ck*P+p]
    base[:, PRED_OFF : PRED_OFF + CK * B * U] = (
        predictor_out.reshape(B, U, CK, P).transpose(3, 2, 0, 1).reshape(P, -1)
    )

    in_maps = []
    for i in range(NCORES):
        m = base.copy()
        enc_s = encoder_out[:, i * TS : (i + 1) * TS, :]  # [b, t, c]
        m[:, ENC_OFF : ENC_OFF + CK * B * TS] = (
            enc_s.reshape(B, TS, CK, P).transpose(3, 2, 0, 1).reshape(P, -1)
        )
        in_maps.append({"cf": m, "cw": wpack})
    return in_maps


def kernel(encoder_out, predictor_out, W, b):
    in_maps = make_in_maps(encoder_out, predictor_out, W, b)
    res = _run(in_maps, trace=False)
    return np.concatenate(
        [np.asarray(res.results[i]["out"], dtype=np.float32) for i in range(NCORES)],
        axis=1,
    )


# revision 4
# speedup vs baseline: 1.3897x; 1.3897x over previous
"""RNN-T Joiner kernel for 8 Trainium2 NeuronCores.

out[b,t,u,:] = tanh(enc[b,t,:] + pred[b,u,:]) @ W.T + b

Sharding: data-parallel over t (400 -> 50 per core). Each core:
  - DVE/GPSIMD: broadcast-add encT[:,t] + predT[:,u] (f32 -> f32; bf16
    outputs would halve the DVE tensor_tensor rate)
  - ACT: tanh f32 -> bf16 logit (the cast rides the activation)
  - PE:  psum[cells, v] += logitT[c, cells].T @ WT[c, v]  (bf16, N=512)
  - output pass per 4-tile psum quad (FD=2048 amortizes the 120-cyc
    PSUM read bubble), two alternating paths so ACT shares the load:
      1-pass: DVE tensor_tensor psum(f32)+bias -> osb bf16   (~2.4us)
      2-pass: ACT quad copy psum->osb bf16 (~1.9us), DVE bf16+bias4_bf
              in place (2x_1P, ~1.2us)
  - DMA: one 500-cell (512KB) store per quad on the sync HWDGE queue
  - host: upcast bf16 -> f32 during the gather

Engine budget per core (cycle-model + measured): PE 640 MM x ~216ns =
~138us critical path; DVE ~116us (20 one-pass quads, 20 bf16 bias adds,
38 producer adds @1.2us), GPSIMD ~97us (42 adds @2.3us), ACT ~119us
(80 tanh @1.02us + 20 quad copies @1.85us), DMA out bf16 20.5MB ~50us.
BLK=5 keeps the startup serial chain (consts DMA -> add -> tanh) ~8us.
"""

import sys

sys.path.insert(0, "/opt/trn_rl_repo")

import ml_dtypes
import numpy as np

import concourse.bass as bass
import concourse.bacc as bacc
import concourse.mybir as mybir
from concourse.tile import TileContext
from concourse.bass_utils import run_bass_kernel_spmd

B, T, U, C, V = 4, 400, 100, 512, 512
NCORES = 8
TS = T // NCORES  # 50 t per core
P = 128
CK = C // P  # 4 chunks of the contraction dim
CELLS = TS * U  # 5000 cells (t,u) per batch per core
BLK = 5  # logit blocks per batch
BCELLS = CELLS // BLK  # 1000
BT = TS // BLK  # 10 t per block
TILE = 125  # uniform cell tile (8 per block); m<128 costs no PE streaming
NT = BCELLS // TILE  # 8
NQ = NT // 2  # tiles per psum quad... actually 4 tiles per quad, 2 quads
F32 = mybir.dt.float32
BF16 = mybir.dt.bfloat16

# f32 consts blob (bias, enc, pred)
BIAS_OFF = 0
ENC_OFF = BIAS_OFF + V  # [ck, b, t]
PRED_OFF = ENC_OFF + CK * B * TS  # [ck, b, u]
NCOL_F = PRED_OFF + CK * B * U  # 2912
# bf16 consts blob: just W, pre-transposed [p, ck, v]
NCOL_W = CK * V  # 2048

N_UNITS = B * BLK * CK  # 80


def _add_engine(unit):
    # 38 units on DVE, 42 on GPSIMD
    if unit % 2 == 0 and unit not in (38, 78):
        return "D"
    return "G"


_cache = {}


def _build():
    # Bacc (not raw Bass): its compile() runs generate_event_semaphores,
    # which splits >1-wait sync conditions that walrus rejects.
    nc = bacc.Bacc("TRN2", target_bir_lowering=False, debug=False)
    cf = nc.declare_dram_parameter("cf", [P, NCOL_F], F32, isOutput=False)
    cw = nc.declare_dram_parameter("cw", [P, NCOL_W], BF16, isOutput=False)
    out = nc.declare_dram_parameter("out", [B, TS, U, V], BF16, isOutput=True)

    with TileContext(nc) as tc:
        with (
            tc.tile_pool(name="consts", bufs=1) as cpool,
            tc.tile_pool(name="arg", bufs=3) as arg_pool,
            tc.tile_pool(name="logit", bufs=3) as logit_pool,
            tc.tile_pool(name="osb", bufs=3) as out_pool,
            tc.tile_pool(name="psum", bufs=2, space="PSUM") as psum_pool,
        ):
            csf = cpool.tile([P, NCOL_F], F32, tag="csf")
            csw = cpool.tile([P, NCOL_W], BF16, tag="csw")
            nc.sync.dma_start(out=csf, in_=cf.ap())
            nc.sync.dma_start(out=csw, in_=cw.ap())

            wview = csw[:].rearrange("p (ck v) -> p ck v", ck=CK)
            bias_f32 = csf[:, BIAS_OFF : BIAS_OFF + V]
            eview = csf[:, ENC_OFF : ENC_OFF + CK * B * TS].rearrange(
                "p (ck b t) -> p ck b t", ck=CK, b=B
            )
            pview = csf[:, PRED_OFF : PRED_OFF + CK * B * U].rearrange(
                "p (ck b u) -> p ck b u", ck=CK, b=B
            )
            bias4_f = bias_f32.unsqueeze(1).broadcast_to([P, 4, V])
            # real (non-broadcast) bf16 bias quad for the 2x_1P in-place add
            bias4_bf = cpool.tile([P, 4, V], BF16, tag="bias4_bf")
            nc.vector.tensor_copy(out=bias4_bf[:], in_=bias4_f)

            quad_idx = 0
            for b in range(B):
                ob = out.ap()[b].rearrange("t u v -> (t u) v")  # [5000, 512]
                for blk in range(BLK):
                    t0 = blk * BT
                    c0 = blk * BCELLS
                    lg = []
                    for ck in range(CK):
                        arg = arg_pool.tile([P, BT, U], F32, tag=f"arg{ck}")
                        lgt = logit_pool.tile([P, BT, U], BF16, tag=f"lg{ck}")
                        e_col = (
                            eview[:, ck, b, t0 : t0 + BT]
                            .unsqueeze(2)
                            .broadcast_to([P, BT, U])
                        )
                        p_row = (
                            pview[:, ck, b, :]
                            .unsqueeze(1)
                            .broadcast_to([P, BT, U])
                        )
                        unit = (b * BLK + blk) * CK + ck
                        eng = nc.vector if _add_engine(unit) == "D" else nc.gpsimd
                        eng.tensor_add(out=arg[:], in0=e_col, in1=p_row)
                        nc.scalar.activation(
                            out=lgt[:],
                            in_=arg[:],
                            func=mybir.ActivationFunctionType.Tanh,
                        )
                        lg.append(lgt[:].rearrange("p t u -> p (t u)"))
                    for q in range(2):
                        ps = psum_pool.tile([P, 4, V], F32, tag="ps")
                        osb = out_pool.tile([P, 4, V], BF16, tag="osb")
                        for j in range(4):
                            s = (q * 4 + j) * TILE
                            for ck in range(CK):
                                nc.tensor.matmul(
                                    ps[:TILE, j, :],
                                    lhsT=lg[ck][:, s : s + TILE],
                                    rhs=wview[:, ck, :],
                                    start=(ck == 0),
                                    stop=(ck == CK - 1),
                                )
                        if quad_idx % 2 == 0:
                            # 2-pass: ACT quad copy, DVE bf16 bias add
                            nc.scalar.activation(
                                out=osb[:TILE],
                                in_=ps[:TILE],
                                func=mybir.ActivationFunctionType.Copy,
                            )
                            nc.vector.tensor_add(
                                out=osb[:TILE],
                                in0=osb[:TILE],
                                in1=bias4_bf[:TILE],
                            )
                        else:
                            # 1-pass: DVE psum+bias (PSUM caps TT at 1x)
                            nc.vector.tensor_add(
                                out=osb[:TILE],
                                in0=ps[:TILE],
                                in1=bias4_f[:TILE],
                            )
                        quad_idx += 1
                        oq = ob[
                            c0 + q * 4 * TILE : c0 + (q + 1) * 4 * TILE
                        ].rearrange("(j p) v -> p j v", p=TILE)
                        nc.sync.dma_start(out=oq, in_=osb[:TILE])
    nc.compile()
    return nc


def _install_ntff_hook():
    """This image's antenv lacks axon_hooks, so bass_utils' trace=True path
    can't find the NTFF profile hook. Inject the module and wire the ctypes
    hook from trn_boot against the axon PJRT .so."""
    if "antenv.axon_hooks" in sys.modules:
        return
    import types

    holder = [None]
    mod = types.ModuleType("antenv.axon_hooks")
    mod.set_axon_ntff_profile_hook = lambda h: holder.__setitem__(0, h)
    mod.get_axon_ntff_profile_hook = lambda: holder[0]
    sys.modules["antenv.axon_hooks"] = mod
    try:
        sys.path.insert(0, "/root/.axon_site/trn_agent_boot")
        from trn_boot import _ntff_profile_via_ctypes

        mod.set_axon_ntff_profile_hook(
            _ntff_profile_via_ctypes("/opt/axon/libaxon_pjrt.so")
        )
    except Exception as e:  # degrade to no tracing
        print(f"NTFF hook install failed: {e}", file=sys.stderr)


def _run(in_maps, trace=False, tmpdir=None):
    if "nc" not in _cache:
        _cache["nc"] = _build()
    if trace:
        _install_ntff_hook()
    return run_bass_kernel_spmd(
        _cache["nc"], in_maps, list(range(NCORES)), trace=trace, tmpdir=tmpdir
    )


def make_in_maps(encoder_out, predictor_out, W, b):
    encoder_out = np.asarray(encoder_out, dtype=np.float32)
    predictor_out = np.asarray(predictor_out, dtype=np.float32)
    W = np.asarray(W, dtype=np.float32)
    b = np.asarray(b, dtype=np.float32)

    # [p, ck, v] <- W[v, ck*P+p]
    wpack = np.ascontiguousarray(
        W.reshape(V, CK, P).transpose(2, 1, 0).reshape(P, CK * V)
    ).astype(ml_dtypes.bfloat16)

    base = np.empty((P, NCOL_F), np.float32)
    base[:, BIAS_OFF : BIAS_OFF + V] = np.broadcast_to(b, (P, V))
    # [p, ck, b, u] <- pred[b, u, ck*P+p]
    base[:, PRED_OFF : PRED_OFF + CK * B * U] = (
        predictor_out.reshape(B, U, CK, P).transpose(3, 2, 0, 1).reshape(P, -1)
    )

    in_maps = []
    for i in range(NCORES):
        m = base.copy()
        enc_s = encoder_out[:, i * TS : (i + 1) * TS, :]  # [b, t, c]
        m[:, ENC_OFF : ENC_OFF + CK * B * TS] = (
            enc_s.reshape(B, TS, CK, P).transpose(3, 2, 0, 1).reshape(P, -1)
        )
        in_maps.append({"cf": m, "cw": wpack})
    return in_maps


def kernel(encoder_out, predictor_out, W, b):
    in_maps = make_in_maps(encoder_out, predictor_out, W, b)
    res = _run(in_maps, trace=False)
    return np.concatenate(
        [np.asarray(res.results[i]["out"], dtype=np.float32) for i in range(NCORES)],
        axis=1,
    )


# revision 8
# speedup vs baseline: 1.5064x; 1.0840x over previous
"""RNN-T Joiner kernel for 8 Trainium2 NeuronCores.

out[b,t,u,:] = tanh(enc[b,t,:] + pred[b,u,:]) @ W.T + b

Sharding: data-parallel over t (400 -> 50 per core). Each core:
  - DVE/GPSIMD: broadcast-add encT[:,t] + predT[:,u] (f32 -> f32; bf16
    outputs would halve the DVE tensor_tensor rate)
  - ACT: tanh f32 -> bf16 logit (the cast rides the activation)
  - PE:  psum[cells, v] += logitT[c, cells].T @ WT[c, v]  (bf16, N=512)
  - output pass per 4-tile psum quad (FD=2048 amortizes the 120-cyc
    PSUM read bubble), two alternating paths so ACT shares the load:
      1-pass: DVE tensor_tensor psum(f32)+bias -> osb bf16   (~2.4us)
      2-pass: ACT quad copy psum->osb bf16 (~1.9us), DVE bf16+bias4_bf
              in place (2x_1P, ~1.2us)
  - DMA: one 500-cell (512KB) store per quad on the sync HWDGE queue
  - host: upcast bf16 -> f32 during the gather

Engine budget per core (cycle-model + measured): PE 640 MM x ~216ns =
~138us critical path; DVE ~116us (20 one-pass quads, 20 bf16 bias adds,
38 producer adds @1.2us), GPSIMD ~97us (42 adds @2.3us), ACT ~119us
(80 tanh @1.02us + 20 quad copies @1.85us), DMA out bf16 20.5MB ~50us.
BLK=5 keeps the startup serial chain (consts DMA -> add -> tanh) ~8us.
"""

import sys

sys.path.insert(0, "/opt/trn_rl_repo")

import ml_dtypes
import numpy as np

import concourse.bass as bass
import concourse.bacc as bacc
import concourse.mybir as mybir
from concourse.tile import TileContext
from concourse.bass_utils import run_bass_kernel_spmd

B, T, U, C, V = 4, 400, 100, 512, 512
NCORES = 8
TS = T // NCORES  # 50 t per core
P = 128
CK = C // P  # 4 chunks of the contraction dim
CELLS = TS * U  # 5000 cells (t,u) per batch per core
BLK = 5  # logit blocks per batch
BCELLS = CELLS // BLK  # 1000
BT = TS // BLK  # 10 t per block
TILE = 125  # uniform cell tile (8 per block); m<128 costs no PE streaming
NT = BCELLS // TILE  # 8
NQ = NT // 2  # tiles per psum quad... actually 4 tiles per quad, 2 quads
F32 = mybir.dt.float32
BF16 = mybir.dt.bfloat16

# f32 consts blob (bias, enc, pred)
BIAS_OFF = 0
ENC_OFF = BIAS_OFF + V  # [ck, b, t]
PRED_OFF = ENC_OFF + CK * B * TS  # [ck, b, u]
NCOL_F = PRED_OFF + CK * B * U  # 2912
# bf16 consts blob: just W, pre-transposed [p, ck, v]
NCOL_W = CK * V  # 2048

N_UNITS = B * BLK * CK  # 80

# producer-add engine per unit: D=DVE tensor_add, G=GPSIMD tensor_add,
# A=ACT fused per-t tanh(p + bias=e_t) (no separate add or tanh).
# Cycle of 10 gives D=24, G=48, A=8 overall; block 0 is hand-spread across
# engines so the first matmul isn't gated on one engine's serial chain.
_PAT = "DGGDGGDGAG"
_UNIT_ENG = [_PAT[u % 10] for u in range(N_UNITS)]
_UNIT_ENG[0:4] = ["D", "G", "A", "D"]


_cache = {}


def _build():
    # Bacc (not raw Bass): its compile() runs generate_event_semaphores,
    # which splits >1-wait sync conditions that walrus rejects.
    nc = bacc.Bacc("TRN2", target_bir_lowering=False, debug=False)
    cf = nc.declare_dram_parameter("cf", [P, NCOL_F], F32, isOutput=False)
    cw = nc.declare_dram_parameter("cw", [P, NCOL_W], BF16, isOutput=False)
    out = nc.declare_dram_parameter("out", [B, TS, U, V], BF16, isOutput=True)

    with TileContext(nc) as tc:
        with (
            tc.tile_pool(name="consts", bufs=1) as cpool,
            tc.tile_pool(name="arg", bufs=3) as arg_pool,
            tc.tile_pool(name="logit", bufs=3) as logit_pool,
            tc.tile_pool(name="osb", bufs=6) as out_pool,
            tc.tile_pool(name="psum", bufs=4, space="PSUM") as psum_pool,
        ):
            csf = cpool.tile([P, NCOL_F], F32, tag="csf")
            csw = cpool.tile([P, NCOL_W], BF16, tag="csw")
            nc.sync.dma_start(out=csf, in_=cf.ap())
            nc.sync.dma_start(out=csw, in_=cw.ap())

            wview = csw[:].rearrange("p (ck v) -> p ck v", ck=CK)
            bias_f32 = csf[:, BIAS_OFF : BIAS_OFF + V]
            eview = csf[:, ENC_OFF : ENC_OFF + CK * B * TS].rearrange(
                "p (ck b t) -> p ck b t", ck=CK, b=B
            )
            pview = csf[:, PRED_OFF : PRED_OFF + CK * B * U].rearrange(
                "p (ck b u) -> p ck b u", ck=CK, b=B
            )
            bias2_f = bias_f32.unsqueeze(1).broadcast_to([P, 2, V])

            for b in range(B):
                ob = out.ap()[b].rearrange("t u v -> (t u) v")  # [5000, 512]
                for blk in range(BLK):
                    t0 = blk * BT
                    c0 = blk * BCELLS
                    lg = []
                    for ck in range(CK):
                        lgt = logit_pool.tile([P, BT, U], BF16, tag=f"lg{ck}")
                        e_col = (
                            eview[:, ck, b, t0 : t0 + BT]
                            .unsqueeze(2)
                            .broadcast_to([P, BT, U])
                        )
                        p_row = (
                            pview[:, ck, b, :]
                            .unsqueeze(1)
                            .broadcast_to([P, BT, U])
                        )
                        unit = (b * BLK + blk) * CK + ck
                        kind = _UNIT_ENG[unit]
                        if kind == "A":
                            # fused add+tanh on ACT, one op per t (bias is
                            # per-partition, fixed per op)
                            for t in range(BT):
                                nc.scalar.activation(
                                    out=lgt[:, t, :],
                                    in_=pview[:, ck, b, :],
                                    func=mybir.ActivationFunctionType.Tanh,
                                    bias=eview[:, ck, b, t0 + t : t0 + t + 1],
                                )
                        else:
                            arg = arg_pool.tile([P, BT, U], F32, tag=f"arg{ck}")
                            eng = nc.vector if kind == "D" else nc.gpsimd
                            eng.tensor_add(out=arg[:], in0=e_col, in1=p_row)
                            nc.scalar.activation(
                                out=lgt[:],
                                in_=arg[:],
                                func=mybir.ActivationFunctionType.Tanh,
                            )
                        lg.append(lgt[:].rearrange("p t u -> p (t u)"))
                    for q in range(NT // 2):
                        ps = psum_pool.tile([P, 2, V], F32, tag="ps")
                        osb = out_pool.tile([P, 2, V], BF16, tag="osb")
                        for j in range(2):
                            s = (q * 2 + j) * TILE
                            for ck in range(CK):
                                nc.tensor.matmul(
                                    ps[:TILE, j, :],
                                    lhsT=lg[ck][:, s : s + TILE],
                                    rhs=wview[:, ck, :],
                                    start=(ck == 0),
                                    stop=(ck == CK - 1),
                                )
                        nc.vector.tensor_add(
                            out=osb[:TILE],
                            in0=ps[:TILE],
                            in1=bias2_f[:TILE],
                        )
                        oq = ob[
                            c0 + q * 2 * TILE : c0 + (q + 1) * 2 * TILE
                        ].rearrange("(j p) v -> p j v", p=TILE)
                        nc.sync.dma_start(out=oq, in_=osb[:TILE])
    nc.compile()
    return nc


def _install_ntff_hook():
    """This image's antenv lacks axon_hooks, so bass_utils' trace=True path
    can't find the NTFF profile hook. Inject the module and wire the ctypes
    hook from trn_boot against the axon PJRT .so."""
    if "antenv.axon_hooks" in sys.modules:
        return
    import types

    holder = [None]
    mod = types.ModuleType("antenv.axon_hooks")
    mod.set_axon_ntff_profile_hook = lambda h: holder.__setitem__(0, h)
    mod.get_axon_ntff_profile_hook = lambda: holder[0]
    sys.modules["antenv.axon_hooks"] = mod
    try:
        sys.path.insert(0, "/root/.axon_site/trn_agent_boot")
        from trn_boot import _ntff_profile_via_ctypes

        mod.set_axon_ntff_profile_hook(
            _ntff_profile_via_ctypes("/opt/axon/libaxon_pjrt.so")
        )
    except Exception as e:  # degrade to no tracing
        print(f"NTFF hook install failed: {e}", file=sys.stderr)


def _run(in_maps, trace=False, tmpdir=None):
    if "nc" not in _cache:
        _cache["nc"] = _build()
    if trace:
        _install_ntff_hook()
    return run_bass_kernel_spmd(
        _cache["nc"], in_maps, list(range(NCORES)), trace=trace, tmpdir=tmpdir
    )


def make_in_maps(encoder_out, predictor_out, W, b):
    encoder_out = np.asarray(encoder_out, dtype=np.float32)
    predictor_out = np.asarray(predictor_out, dtype=np.float32)
    W = np.asarray(W, dtype=np.float32)
    b = np.asarray(b, dtype=np.float32)

    # [p, ck, v] <- W[v, ck*P+p]
    wpack = np.ascontiguousarray(
        W.reshape(V, CK, P).transpose(2, 1, 0).reshape(P, CK * V)
    ).astype(ml_dtypes.bfloat16)

    base = np.empty((P, NCOL_F), np.float32)
    base[:, BIAS_OFF : BIAS_OFF + V] = np.broadcast_to(b, (P, V))
    # [p, ck, b, u] <- pred[b, u, ck*P+p]
    base[:, PRED_OFF : PRED_OFF + CK * B * U] = (
        predictor_out.reshape(B, U, CK, P).transpose(3, 2, 0, 1).reshape(P, -1)
    )

    in_maps = []
    for i in range(NCORES):
        m = base.copy()
        enc_s = encoder_out[:, i * TS : (i + 1) * TS, :]  # [b, t, c]
        m[:, ENC_OFF : ENC_OFF + CK * B * TS] = (
            enc_s.reshape(B, TS, CK, P).transpose(3, 2, 0, 1).reshape(P, -1)
        )
        in_maps.append({"cf": m, "cw": wpack})
    return in_maps


def kernel(encoder_out, predictor_out, W, b):
    in_maps = make_in_maps(encoder_out, predictor_out, W, b)
    res = _run(in_maps, trace=False)
    return np.concatenate(
        [np.asarray(res.results[i]["out"], dtype=np.float32) for i in range(NCORES)],
        axis=1,
    )
